# revision 37
# baseline (speedup 1.0000x reference)
"""Trainium2 Bass kernel for nn_CaTentLayer (depthwise temporal tent-filter conv).

Reference computation (T=16384 time bins, NC=1024 cells, FW=128 filter taps):
    wnorm = max(||W[:,c]||_2, 1e-8)
    filt  = max(0, W / wnorm)                       # [FW, NC]
    pre[t,c] = sum_k x[t+k-63, c] * filt[k, c]      # depthwise conv, SAME pad
    out = roll(pre + b, shift=64, axis=0)           # circular roll on time

Sharding: NC (cell) axis across 8 cores, 128 channels each — fully
independent, no collectives.

Per-channel algorithm (TensorEngine): tile time as t = 128n + u and tile x
with a -63 offset, X1[v, m] = x(128m - 63 + v) (zero padded). Because
63 + 64 = FW - 1, the 255-sample support of each output tile spans exactly
two adjacent X1 tiles:
    pre_tile[u, n] = A^T @ X1[:, n] + B^T @ X1[:, n+1]
    A[v, u] = filt[v - u]        for v >= u   (lower triangle incl diag)
    B[v, u] = filt[v - u + 128]  for v <  u   (strict upper triangle)
A and B are complementary triangles of ONE packed circulant
    PACK0[v, u] = filt[(v - u) mod 128, c]          (fp16, 4 MB/core)
so the host ships PACK0 and the device splits it with two constant 0/1
mask multiplies on the Vector engine (16 channels per instruction; A is
masked in place, B goes to a second buffer). Two full-range fp16 matmuls
per channel accumulate into fp32 PSUM with clean start/stop groups.

PSUM->SBUF evacuation (+ bias) is split between the Scalar and Vector
engines to balance engine occupancy; the circular roll is folded into the
output DMA offsets. x and the packed filter stream in 16-channel chunks at
full DMA rate so the first matmul starts within a few microseconds.
"""

import numpy as np

import concourse.bacc as bacc
import concourse.mybir as mybir
import concourse.tile as tile
from concourse.bass_utils import run_bass_kernel_spmd

T = 16384
NC = 1024
FW = 128
N_CORES = 8
CS = NC // N_CORES  # 128 channels per core
P = 128             # partitions / time-tile size
NT = T // P         # 128 time tiles
CG = 16             # channels per split/load instruction
F32 = mybir.dt.float32
F16 = mybir.dt.float16
NP16 = np.float16

_CACHE: dict = {}


def _build_bass(repeat: int = 1):
    nc = bacc.Bacc("TRN2", target_bir_lowering=False, debug=False,
                   num_devices=N_CORES)

    # xs: x tiled with -63 offset, [v, c, m], m in [0, NT], fp16, c-major
    xs_d = nc.dram_tensor("xs", [P, CS, NT + 1], F16, kind="ExternalInput")
    # tbp: packed circulant blocks PACK0, v-major [v, c, u], fp16
    tbp_d = nc.dram_tensor("tbp", [P, CS, P], F16, kind="ExternalInput")
    # mk: split masks [v, {B-upper, A-lower}, u], fp16
    mk_d = nc.dram_tensor("mk", [P, 2, P], F16, kind="ExternalInput")
    bb_d = nc.dram_tensor("bb", [P, CS], F32, kind="ExternalInput")
    out_d = nc.dram_tensor("out", [T, CS], F32, kind="ExternalOutput")

    ident = mybir.ActivationFunctionType.Identity

    with tile.TileContext(nc) as tc:
        with (
            tc.tile_pool(name="xbuf", bufs=1) as xpool,
            tc.tile_pool(name="pk", bufs=1) as pkpool,
            tc.tile_pool(name="mp", bufs=1) as mppool,
            tc.tile_pool(name="obuf", bufs=1) as opool,
            tc.tile_pool(name="misc", bufs=1) as mpool,
            tc.tile_pool(name="ps", bufs=8, space="PSUM") as pspool,
        ):
          for _rep in range(repeat):
            X = xpool.tile([P, CS, NT + 1], F16)     # [v, c, m]
            PK = pkpool.tile([P, CS, P], F16)        # PACK0 -> A in place
            MP = mppool.tile([P, CS, P], F16)        # B (strict upper)
            MK = mpool.tile([P, 2, P], F16, tag="mk")
            B = mpool.tile([P, CS], F32, tag="b")

            # interleaved chunked loads + splits so channel-group 0 is ready
            # within a few us and the rest streams behind the matmuls; the
            # first groups are small to cut the prologue, and the bias load
            # is deferred past them (first use is the first evacuation)
            groups = [(0, 4), (4, 4), (8, 8), (16, 16), (32, 16), (48, 16),
                      (64, 16), (80, 16), (96, 16), (112, 16)]
            for gi, (c0, gw) in enumerate(groups):
                sl = slice(c0, c0 + gw)
                nc.sync.dma_start(X[:, sl, :], xs_d[:, sl, :])
                nc.sync.dma_start(PK[:, sl, :], tbp_d[:, sl, :])
                if gi == 0:
                    nc.sync.dma_start(MK[:], mk_d[:])
                if gi == 2:
                    nc.sync.dma_start(B[:], bb_d[:])
                mk_b = MK[:, 0, :].unsqueeze(1).broadcast_to([P, gw, P])
                mk_a = MK[:, 1, :].unsqueeze(1).broadcast_to([P, gw, P])
                nc.vector.tensor_mul(MP[:, sl, :], PK[:, sl, :], mk_b)
                nc.vector.tensor_mul(PK[:, sl, :], PK[:, sl, :], mk_a)

            # two full-range conv matmuls per channel; evacuation (+bias)
            # split between Scalar and Vector engines
            O = opool.tile([P, NT, CS], F32)
            for c in range(CS):
                ps = pspool.tile([P, NT], F32, tag="ps")
                nc.tensor.matmul(ps[:, :], PK[:, c, :], X[:, c, 0:NT],
                                 start=True, stop=False)
                nc.tensor.matmul(ps[:, :], MP[:, c, :], X[:, c, 1:NT + 1],
                                 start=False, stop=True)
                if c % 4 == 3:
                    nc.vector.tensor_scalar_add(O[:, :, c], ps[:],
                                                B[:, c:c + 1])
                else:
                    nc.scalar.activation(O[:, :, c], ps[:], ident,
                                         bias=B[:, c:c + 1])

            # store with roll(+64): t = 128n + v -> row t+64 (mod T)
            nc.sync.dma_start(
                out_d[64:T - 64, :].rearrange("(n v) c -> v n c", v=P),
                O[:, 0:NT - 1, :])
            nc.sync.dma_start(out_d[T - 64:T, :], O[0:64, NT - 1, :])
            nc.sync.dma_start(out_d[0:64, :], O[64:P, NT - 1, :])

    nc.compile()
    return nc


def _host_prep(x, W, b):
    """Per-core input maps: -63-offset tiled fp16 x shard (c-major), packed
    circulant filter blocks (fp16), split masks, broadcast bias."""
    x = np.asarray(x, dtype=np.float32)
    W = np.asarray(W, dtype=np.float32)
    b = np.asarray(b, dtype=np.float32)

    wnorm = np.maximum(np.sqrt((W * W).sum(axis=0)), np.float32(1e-8))
    filt = np.maximum(np.float32(0.0), W / wnorm)          # [FW, NC]

    v = np.arange(P)
    u = np.arange(P)
    d = v[:, None] - u[None, :]
    pack0 = filt[d % 128, :].astype(NP16)                  # [v, u, NC]
    masks = np.stack([
        (d < 0),    # B: strict upper triangle
        (d >= 0),   # A: lower triangle incl diagonal
    ]).astype(NP16)                                        # [2, v, u]
    masks_vju = np.ascontiguousarray(masks.transpose(1, 0, 2))  # [v, 2, u]

    # xpad[63 + t] = x[t]; length 63 + T + 65 = 129*128 exactly
    xpad = np.zeros(((NT + 1) * P, NC), NP16)
    xpad[63:63 + T] = x.astype(NP16)
    xt = xpad.reshape(NT + 1, P, NC)                       # [m, v, NC]

    in_maps = []
    for g in range(N_CORES):
        sl = slice(g * CS, (g + 1) * CS)
        xs = np.ascontiguousarray(xt[:, :, sl].transpose(1, 2, 0))  # [v,c,m]
        tbp = np.ascontiguousarray(pack0[:, :, sl].transpose(0, 2, 1))
        bb = np.ascontiguousarray(
            np.broadcast_to(b[sl][None, :], (P, CS)))
        in_maps.append({"xs": xs, "tbp": tbp, "mk": masks_vju, "bb": bb})
    return in_maps


def kernel(x: np.ndarray, W: np.ndarray, b: np.ndarray) -> np.ndarray:
    if "nc" not in _CACHE:
        _CACHE["nc"] = _build_bass()
    nc = _CACHE["nc"]
    in_maps = _host_prep(x, W, b)
    res = run_bass_kernel_spmd(nc, in_maps, core_ids=list(range(N_CORES)))
    return np.concatenate([res.results[g]["out"] for g in range(N_CORES)],
                          axis=1)
